# revision 2
# baseline (speedup 1.0000x reference)
"""Banded (sliding-window) multi-head attention on 8 Trainium2 NeuronCores.

Problem: B=2, S=2048, D=512, H=8 heads (hd=64), window=256 (|i-j| <= 128),
  qkv = x @ Wqkv + bqkv           -> per-head q,k,v
  scores = (q k^T masked to band) / 8 ; softmax ; out = (attn v) @ Wo + bo

Sharding: core = (batch b in {0,1}) x (head-group g in {0..3}); each core
computes 2 heads over the full sequence of one batch element plus the o_proj
partial product for its heads' embed slice. The host sums the 4 partials per
batch and adds bo.

Design notes (v2, evolved from the 38811ns bf16 kernel):
  - The qkv projection runs as fp8e4m3 DoubleRow matmuls with residual
    compensation: x ~ X0+X1 and 32*Wqkv ~ W0+W1 (fp8 value + unscaled fp8
    residual), accumulating X0W0 + X0W1 + X1W0 in PSUM.  DoubleRow contracts
    2 partition-slots per cycle at 0.5 cycles/col, so the 512-deep qkv
    contraction runs 4x faster per term than bf16; 3 terms -> 1.5x net PE
    savings at bf16-level accuracy (residuals capture the fp8 rounding).
    The x32 weight scale keeps W out of the fp8 subnormal range; q,k come
    out 32x hot each so the exp scale drops to 0.125/1024 = 2^-13, v is 32x
    hot so the host divides the final output by 32.
  - Scores / AV / o_proj matmuls stay bf16 (fp8 anywhere post-projection
    measured 2-3e-2 end-to-end error vs the 2e-2 gate).
  - Both heads' score blocks land in the two banks of one [128,2,512]
    PSUM tile so a single exp op covers them; the band masks multiply only
    the two 128-col edge triangles of the window, both heads + both edges
    in ONE strided DVE/Pool op per key block (tm broadcast along h).
  - V is projected straight into natural [key, v0|ones|v1|ones] layout on
    the PE (out = x_sliceT.T @ Wv), so the AV matmul's 128-wide stationary
    [v_h | ones] emits values (rows 0:64) AND 64 identical denominator
    rows (64:128) in one pass: softmax normalize = one DVE reciprocal
    plus one fused PSUM*SBUF->bf16 multiply.
  - Software-pipelined emission: chunk c's AV/normalize/o_proj spread over
    the next chunk's score slots; the final chunk is processed in two
    256-query halves so only a short half-width chain trails the last exp.
  - Dependency-free warmup matmuls hold the PE p-state through the DMA
    ramp; cc0's Q/K run as 256-col halves so the first exp fires early;
    bulk loads ride the SWDGE ring (prep on the idle-early Pool engine).
  - GPSIMD (Pool) cannot touch PSUM, so PSUM reads split between ACT
    (exp, some copies) and DVE (copies, reciprocal, normalize); the
    SBUF-only band masks go to DVE (bf16 2x mode) and Pool.
"""

import numpy as np
import ml_dtypes

import concourse.bass as bass  # noqa: F401  (engine types via nc)
import concourse.mybir as mybir
import concourse.tile as tile
from concourse import bacc
from concourse.bass_utils import run_bass_kernel_spmd

B, S, DIN, E = 2, 2048, 512, 512
H, HD = 8, 64
NB = S // 128      # 16 key/query blocks of 128
F32 = mybir.dt.float32
F32R = mybir.dt.float32r
BF16 = mybir.dt.bfloat16
FP8 = mybir.dt.float8e4
EXPF = mybir.ActivationFunctionType.Exp
IDENT = mybir.ActivationFunctionType.Identity
DR = mybir.MatmulPerfMode.DoubleRow
NPBF = ml_dtypes.bfloat16
NP8 = ml_dtypes.float8_e4m3
EXP_SCALE = 0.125 / 1024.0  # 2^-13: band scores carry a 32x32 weight scale

# kb blocks whose 3-block score window is covered once qkv chunk cc is done
KB_GROUPS = {0: [0, 1, 2], 1: [3, 4, 5, 6], 2: [7, 8, 9, 10],
             3: [11, 12, 13, 14, 15]}
# (x residual idx, w residual idx): X0W0 + X0W1 + X1W0 (X1W1 dropped)
TERMS = [(0, 0), (0, 1), (1, 0)]

_CACHE = {}
LAST_RESULTS = None  # BassKernelResults of the most recent run (for test.py)


def _build_nc():
    nc = bacc.Bacc(None, target_bir_lowering=False, debug=False)

    xt0 = nc.dram_tensor("xt0", [4, DIN, 512], FP8, kind="ExternalInput")
    xt1 = nc.dram_tensor("xt1", [4, DIN, 512], FP8, kind="ExternalInput")
    wq0 = nc.dram_tensor("wq0", [128, 4, 384], FP8, kind="ExternalInput")
    wq1 = nc.dram_tensor("wq1", [128, 4, 384], FP8, kind="ExternalInput")
    wo = nc.dram_tensor("wo", [128, E], BF16, kind="ExternalInput")
    km = nc.dram_tensor("km", [128, NB], F32, kind="ExternalInput")
    tm = nc.dram_tensor("tm", [128, 384], BF16, kind="ExternalInput")
    outt = nc.dram_tensor("outt", [E, S], BF16, kind="ExternalOutput")

    with tile.TileContext(nc) as tc:
        with (
            tc.tile_pool(name="sb", bufs=1) as sb,
            tc.tile_pool(name="ps_qkv", bufs=2, space="PSUM") as ps_qkv,
            tc.tile_pool(name="ps_st", bufs=2, space="PSUM") as ps_st,
            tc.tile_pool(name="ps_ot", bufs=2, space="PSUM") as ps_ot,
            tc.tile_pool(name="small", bufs=4) as small,
        ):
            xt0_sb = sb.tile([128, 4, 4, 512], FP8)   # [p, kc, cc, q]
            xt1_sb = sb.tile([128, 4, 4, 512], FP8)
            wq0_sb = sb.tile([128, 4, 384], FP8)      # [p, kc, fo]
            wq1_sb = sb.tile([128, 4, 384], FP8)
            wo_sb = sb.tile([128, E], BF16)
            km_sb = sb.tile([128, NB], F32)
            tm2_sb = sb.tile([128, 384], BF16)
            qkvt = sb.tile([128, 2, S], BF16)         # fb0=Q, fb1=K (h0|h1)
            # V natural [key, v0|ones|v1|ones]: per head, [v_h | ones] is a
            # contiguous 128-col stationary read, so one AV matmul emits
            # values (rows 0:64) and denominator copies (rows 64:128)
            vboth = sb.tile([128, NB, 256], BF16)
            valst = sb.tile([128, S], BF16)           # normalized attn @ V
            outt_sb = sb.tile([128, 4, S], BF16)
            p_sb = sb.tile([128, NB, 2, 384], BF16)   # exp'd scores, h-minor

            # HWDGE descriptor generation (~630ns/op) serializes the input
            # stream, so only the launch-critical cc0 activations use it; the
            # weights, masks and later x chunks ride the SWDGE ring whose
            # descriptor prep runs on the idle-early Pool engine
            nc.gpsimd.dma_start(out=wq0_sb, in_=wq0[:, :, :])
            nc.gpsimd.dma_start(out=wq1_sb, in_=wq1[:, :, :])
            nc.gpsimd.dma_start(out=km_sb, in_=km[:, :])
            nc.gpsimd.dma_start(out=tm2_sb, in_=tm[:, :])
            for half in range(2):
                nc.sync.dma_start(
                    out=xt0_sb[:, 2 * half:2 * half + 2, 0, :],
                    in_=xt0[0, half * 256:(half + 1) * 256, :]
                    .rearrange("(kc p) q -> p kc q", p=128),
                )
                nc.scalar.dma_start(
                    out=xt1_sb[:, 2 * half:2 * half + 2, 0, :],
                    in_=xt1[0, half * 256:(half + 1) * 256, :]
                    .rearrange("(kc p) q -> p kc q", p=128),
                )
            for cc in range(1, 3):
                nc.sync.dma_start(
                    out=xt0_sb[:, :, cc, :],
                    in_=xt0[cc].rearrange("(kc p) q -> p kc q", p=128),
                )
                nc.gpsimd.dma_start(
                    out=xt1_sb[:, :, cc, :],
                    in_=xt1[cc].rearrange("(kc p) q -> p kc q", p=128),
                )
            nc.gpsimd.dma_start(
                out=xt0_sb[:, :, 3, :],
                in_=xt0[3].rearrange("(kc p) q -> p kc q", p=128),
            )
            nc.gpsimd.dma_start(
                out=xt1_sb[:, :, 3, :],
                in_=xt1[3].rearrange("(kc p) q -> p kc q", p=128),
            )
            nc.sync.dma_start(out=wo_sb, in_=wo[:, :])

            # denominator ones blocks at cols 64:128 and 192:256 of every key
            # block, so AV emits 64 identical denominator rows (rows 64:128
            # of ot) and the normalize can read the reciprocal per-partition
            nc.gpsimd.memset(vboth[:, :, 64:128], 1.0)
            nc.gpsimd.memset(vboth[:, :, 192:256], 1.0)

            # PE p-state warm-up: dependency-free garbage matmuls keep the
            # tensor engine continuously busy through the DMA ramp so the
            # first real qkv matmuls run at the full (not half) clock
            wu_in = sb.tile([128, 512], BF16)
            nc.vector.memset(wu_in, 0.0)
            # dummy exp at t~0 hoists the 1.28us Exp table load into the DMA
            # ramp; without it the load lands right before the first real exp
            dummy = small.tile([1, 16], BF16, tag="rc", name="dummy")
            nc.scalar.activation(out=dummy, in_=wu_in[0:1, 0:16], func=EXPF)
            wu = ps_qkv.tile([128, 512], F32, tag="qkv", name="wu")
            for _ in range(8):
                nc.tensor.matmul(
                    wu,
                    wu_in[:, 0:128],
                    wu_in,
                    start=True,
                    stop=True,
                    skip_group_check=True,
                )

            def dr_group(ps_ap, stat_of, mov_of, skip_group_check=False):
                # accumulate the 3-term fp8 residual product over kc pairs
                n = 0
                for i in range(2):
                    for (xi, wi) in TERMS:
                        n += 1
                        nc.tensor.matmul(
                            ps_ap,
                            stat_of(i, xi, wi),
                            mov_of(i, xi, wi),
                            start=(n == 1),
                            stop=(n == 6),
                            perf_mode=DR,
                            skip_group_check=skip_group_check,
                        )

            def qkv_fb(cc, fb):
                # qkvT = Wg^T @ x[b]^T for query chunk cc, feature block fb
                ps = ps_qkv.tile([128, 512], F32, tag="qkv", name="ps")
                dr_group(
                    ps,
                    lambda i, xi, wi: (wq0_sb, wq1_sb)[wi]
                    [:, 2 * i:2 * i + 2, fb * 128:(fb + 1) * 128],
                    lambda i, xi, wi: (xt0_sb, xt1_sb)[xi]
                    [:, 2 * i:2 * i + 2, cc, :],
                )
                if cc == 1:
                    nc.vector.tensor_copy(
                        qkvt[:, fb, cc * 512:(cc + 1) * 512], ps)
                else:
                    nc.scalar.activation(
                        out=qkvt[:, fb, cc * 512:(cc + 1) * 512],
                        in_=ps, func=IDENT)

            def qkv_fb_half(cc, fb, half):
                # 256-col half of a Q/K feature block (ramp shortener)
                o = half * 256
                ps = ps_qkv.tile([128, 256], F32, tag="qkv", name="psh")
                dr_group(
                    ps,
                    lambda i, xi, wi: (wq0_sb, wq1_sb)[wi]
                    [:, 2 * i:2 * i + 2, fb * 128:(fb + 1) * 128],
                    lambda i, xi, wi: (xt0_sb, xt1_sb)[xi]
                    [:, 2 * i:2 * i + 2, cc, o:o + 256],
                )
                nc.vector.tensor_copy(
                    qkvt[:, fb, cc * 512 + o:cc * 512 + o + 256], ps)

            def qkv_v(cc):
                # V in natural [key, vcol] layout straight off the PE:
                # out[key, v] = x^T[kc, key] @ Wv[kc, v], one 128-key block
                # per bank quarter, both heads in the 128 v-columns
                ps = ps_qkv.tile([128, 4, 128], F32, tag="qkv", name="psv")
                for j in range(4):
                    dr_group(
                        ps[:, j, :],
                        lambda i, xi, wi: (xt0_sb, xt1_sb)[xi]
                        [:, 2 * i:2 * i + 2, cc, j * 128:(j + 1) * 128],
                        lambda i, xi, wi: (wq0_sb, wq1_sb)[wi]
                        [:, 2 * i:2 * i + 2, 256:384],
                        skip_group_check=True,
                    )
                # v0 -> cols 0:64, v1 -> cols 128:192 of each key block
                dst = (vboth[:, 4 * cc:4 * cc + 4, :]
                       .rearrange("p n (g c) -> p n g c", c=64)[:, :, 0:3:2, :])
                src = ps.rearrange("p n (g c) -> p n g c", c=64)
                nc.vector.tensor_copy(dst, src)

            def scores_block(kb):
                # both heads' score blocks land in the two banks of one st
                # tile so a single exp op covers them
                ws, we = max(0, kb - 1), min(NB - 1, kb + 1)
                nq = (we - ws + 1) * 128
                st = ps_st.tile([128, 2, 512], F32, tag="st", name="st")
                for h in range(2):
                    hp = 64 * h
                    nc.tensor.matmul(
                        st[:, h, :nq],
                        qkvt[hp:hp + 64, 1, kb * 128:(kb + 1) * 128],
                        qkvt[hp:hp + 64, 0, ws * 128:(we + 1) * 128],
                        start=True,
                        stop=True,
                        skip_group_check=True,
                    )
                nc.scalar.activation(
                    out=p_sb[:, kb, 0:2, 0:nq],
                    in_=st[:, 0:2, 0:nq],
                    func=EXPF,
                    bias=km_sb[:, kb:kb + 1],
                    scale=EXP_SCALE,
                )
                # band mask: only the two 128-col edge triangles of the
                # window need zeroing (the center block is all-ones); ONE
                # strided op covers both heads (and both edges for middle
                # blocks).  Slack-rich masks go to the otherwise-idle
                # GPSIMD engine; chunk-edge and tail-critical masks on DVE.
                eng = (nc.gpsimd
                       if ((kb >= 5 and kb % 4 in (1, 2, 3) and kb != 15) or kb <= 2)
                       else nc.vector)
                if kb == 0:           # window [ones | j>=i]
                    pe_ap = p_sb[:, kb, 0:2, 128:256]
                    tm_ap = (tm2_sb[:, None, 256:384]
                             .broadcast_to([128, 2, 128]))
                elif kb == NB - 1:    # window [j<=i | ones]
                    pe_ap = p_sb[:, kb, 0:2, 0:128]
                    tm_ap = (tm2_sb[:, None, 0:128]
                             .broadcast_to([128, 2, 128]))
                else:                 # [j<=i | ones | j>=i]
                    pe_ap = (p_sb[:, kb, 0:2, 0:384]
                             .rearrange("p h (e c) -> p h e c", c=128)
                             [:, :, 0:3:2, :])
                    tm_ap = (tm2_sb.rearrange("p (e c) -> p e c", c=128)
                             [:, None, 0:3:2, :]
                             .broadcast_to([128, 2, 2, 128]))
                eng.tensor_mul(pe_ap, pe_ap, tm_ap)

            ot_live = {}

            def av_mm(h, qblo, qbhi, kbs, start, stop, alloc=False):
                # accumulate attn @ [V|ones] for query blocks [qblo, qbhi]
                # into ot(h); ot column 0 = query block qblo
                if alloc:
                    ot_live[h] = (ps_ot.tile([128, 512], F32, tag="ot",
                                             name="ot"), qblo)
                ot, qb0 = ot_live[h]
                for i, kb in enumerate(kbs):
                    ws, we = max(0, kb - 1), min(NB - 1, kb + 1)
                    qs, qe = max(ws, qblo), min(we, qbhi)
                    if qs > qe:
                        continue
                    nc.tensor.matmul(
                        ot[:, (qs - qb0) * 128:(qe + 1 - qb0) * 128],
                        vboth[:, kb, 128 * h:128 * h + 128],
                        p_sb[:, kb, h, (qs - ws) * 128:(qe + 1 - ws) * 128],
                        start=start and i == 0,
                        stop=stop and i == len(kbs) - 1,
                        skip_group_check=True,
                    )

            def av_recip(h, qblo, qbhi):
                # reciprocal of the denominator rows (64 identical copies in
                # rows 64:128 of ot)
                ot, qb0 = ot_live[h]
                lo, hi = (qblo - qb0) * 128, (qbhi + 1 - qb0) * 128
                rcb = small.tile([64, 512], F32R, tag="rc", name="rcb")
                with nc.allow_low_precision("f32r softmax denom recip"):
                    nc.vector.reciprocal(rcb[:, 0:hi - lo], ot[64:128, lo:hi])
                ot_live[h] = (ot, qb0, rcb)

            def av_norm(h, qblo, qbhi, pop=False):
                hp = 64 * h
                ot, qb0, rcb = ot_live.pop(h) if pop else ot_live[h]
                lo, hi = (qblo - qb0) * 128, (qbhi + 1 - qb0) * 128
                nc.vector.tensor_mul(
                    valst[hp:hp + 64, qblo * 128:(qbhi + 1) * 128],
                    ot[0:64, lo:hi],
                    rcb[:, 0:hi - lo],
                )

            def oproj_range(qblo, qbhi, tail=False, late=False):
                lo, hi = qblo * 128, (qbhi + 1) * 128
                w = hi - lo
                if not tail:
                    for fo in range(4):
                        po = ps_qkv.tile([128, 512], F32, tag="qkv",
                                         name="po")
                        nc.tensor.matmul(
                            po[:, 0:w],
                            wo_sb[:, fo * 128:(fo + 1) * 128],
                            valst[:, lo:hi],
                            start=True,
                            stop=True,
                        )
                        if fo in (0, 1):
                            nc.scalar.activation(
                                out=outt_sb[:, fo, lo:hi],
                                in_=po[:, 0:w], func=IDENT)
                        else:
                            nc.vector.tensor_copy(outt_sb[:, fo, lo:hi],
                                                  po[:, 0:w])
                        nc.sync.dma_start(
                            out=outt[fo * 128:(fo + 1) * 128, lo:hi],
                            in_=outt_sb[:, fo, lo:hi],
                        )
                    return
                # tail chunks borrow the (now idle) st pool's 2-bank tiles:
                # four independent po slots kill the mm->copy->slot-free
                # serialization, and pair-merged copies/writes halve the op
                # count on the drain
                for pair in range(2):
                    po2 = ps_st.tile([128, 2, 512], F32, tag="st",
                                     name="po2")
                    for k in range(2):
                        fo = 2 * pair + k
                        nc.tensor.matmul(
                            po2[:, k, 0:w],
                            wo_sb[:, fo * 128:(fo + 1) * 128],
                            valst[:, lo:hi],
                            start=True,
                            stop=True,
                            skip_group_check=True,
                        )
                    f0 = 2 * pair
                    if not late or pair == 0:
                        nc.scalar.activation(
                            out=outt_sb[:, f0:f0 + 2, lo:hi],
                            in_=po2[:, :, 0:w], func=IDENT)
                    else:
                        nc.vector.tensor_copy(outt_sb[:, f0:f0 + 2, lo:hi],
                                              po2[:, :, 0:w])
                    ring = nc.sync if pair == 0 else nc.scalar
                    ring.dma_start(
                        out=outt.rearrange("(f p) s -> p f s", p=128)
                        [:, f0:f0 + 2, lo:hi],
                        in_=outt_sb[:, f0:f0 + 2, lo:hi],
                    )

            # chunk c's AV window closes at kb=4c+4; its AV/normalize/o_proj
            # work is spread over kb=4c+5..4c+7 so PE keeps feeding the ACT
            # exp stream with scores while the boundary chain drains.  The
            # final chunk is processed in two 256-query halves: the first
            # half's window closes at kb=14, so only the second half-width
            # chain remains after the last exp.
            def kbs_for(c):
                return list(range(max(0, 4 * c - 1), min(NB - 1, 4 * c + 4) + 1))

            for cc in range(4):
                if cc == 0:
                    # first chunk's Q/K in 256-col halves: the first exp only
                    # needs Q blocks 0-1 / K block 0, so fire it early
                    qkv_fb_half(0, 0, 0)
                    qkv_fb_half(0, 1, 0)
                else:
                    qkv_fb(cc, 0)
                    qkv_fb(cc, 1)
                for ikb, kb in enumerate(KB_GROUPS[cc]):
                    scores_block(kb)
                    if ikb == 0:
                        # V projection slots in after the group's first
                        # scores so the exp stream isn't starved at the
                        # chunk boundary
                        if cc == 0:
                            qkv_fb_half(0, 0, 1)
                            qkv_fb_half(0, 1, 1)
                        qkv_v(cc)
                    c = kb // 4 - 1
                    if c >= 0 and c < 3:
                        if kb % 4 == 1:
                            av_mm(0, 4 * c, 4 * c + 3, kbs_for(c),
                                  True, True, alloc=True)
                            av_recip(0, 4 * c, 4 * c + 3)
                        elif kb % 4 == 2:
                            av_norm(0, 4 * c, 4 * c + 3, pop=True)
                            av_mm(1, 4 * c, 4 * c + 3, kbs_for(c),
                                  True, True, alloc=True)
                            av_recip(1, 4 * c, 4 * c + 3)
                        elif kb % 4 == 3:
                            av_norm(1, 4 * c, 4 * c + 3, pop=True)
                            oproj_range(4 * c, 4 * c + 3, tail=(c == 2))
                    if kb == 14:
                        # chunk-3 first half (qb 12-13): window closes here
                        av_mm(0, 12, 13, [11, 12, 13, 14], True, True,
                              alloc=True)
                        av_recip(0, 12, 13)
                    elif kb == 15:
                        av_norm(0, 12, 13, pop=True)
                        av_mm(1, 12, 13, [11, 12, 13, 14], True, True,
                              alloc=True)
                        av_recip(1, 12, 13)
            # tail: first-half norm + oproj overlap the second half's chain
            av_norm(1, 12, 13, pop=True)
            av_mm(0, 14, 15, [13, 14, 15], True, True, alloc=True)
            av_recip(0, 14, 15)
            oproj_range(12, 13, tail=True)
            av_norm(0, 14, 15, pop=True)
            av_mm(1, 14, 15, [13, 14, 15], True, True, alloc=True)
            av_recip(1, 14, 15)
            av_norm(1, 14, 15, pop=True)
            oproj_range(14, 15, tail=True, late=True)

    nc.finalize()
    return nc


def _numpy_reference(x, padding_mask, Wqkv, bqkv, Wo, bo):
    """Fallback for input regimes the device path does not cover."""
    b, s, _ = x.shape
    qkv = x @ Wqkv + bqkv
    qkv = qkv.reshape(b, s, H, 3 * HD).transpose(0, 2, 1, 3)
    q, k, v = np.split(qkv, 3, axis=-1)
    scores = np.einsum("bhqd,bhkd->bhqk", q, k)
    idx = np.arange(s)
    band = np.abs(idx[:, None] - idx[None, :]) <= 128
    pm = padding_mask != 0
    valid = band[None, None] & pm[:, None, None, :] & pm[:, None, :, None]
    scores = np.where(valid, scores, -np.inf) / np.sqrt(HD)
    scores = scores - scores.max(axis=-1, keepdims=True)
    with np.errstate(invalid="ignore", over="ignore"):
        e = np.exp(scores)
        attn = e / e.sum(axis=-1, keepdims=True)
    attn = np.nan_to_num(attn, nan=0.0)
    vals = np.einsum("bhqk,bhkd->bhqd", attn, v)
    vals = vals.transpose(0, 2, 1, 3).reshape(b, s, E)
    return (vals @ Wo + bo).astype(np.float32)


def kernel(x, padding_mask, Wqkv, bqkv, Wo, bo):
    global LAST_RESULTS
    x = np.ascontiguousarray(np.asarray(x, np.float32))
    Wqkv = np.asarray(Wqkv, np.float32)
    bqkv = np.asarray(bqkv, np.float32)
    Wo = np.asarray(Wo, np.float32)
    bo = np.asarray(bo, np.float32)
    pm = np.asarray(padding_mask)

    if np.any(bqkv != 0):
        # qkv bias is identically zero in the target problem; the device
        # program folds no qkv bias, so fall back rather than be wrong.
        return _numpy_reference(x, pm, Wqkv, bqkv, Wo, bo)

    if "nc" not in _CACHE:
        _CACHE["nc"] = _build_nc()
    nc = _CACHE["nc"]

    # trimask [key p, 384]: window cols = [qb-1 | qb | qb+1] relative blocks
    j = np.arange(128)[:, None]
    i = np.arange(128)[None, :]
    tm = np.concatenate(
        [(j <= i), np.ones((128, 128), bool), (j >= i)], axis=1
    ).astype(NPBF)

    in_maps = []
    for core in range(8):
        b, g = divmod(core, 4)
        # feature permutation for this head group: [q0|q1|k0|k1|v0|v1]
        h0, h1 = 2 * g, 2 * g + 1
        cols = []
        for kind in range(3):  # q, k, v
            for h in (h0, h1):
                base = h * 3 * HD + kind * HD
                cols.extend(range(base, base + HD))
        wq_g = Wqkv[:, cols] * 32.0                           # [512, 384]
        w0 = wq_g.astype(NP8)
        w1 = (wq_g - w0.astype(np.float32)).astype(NP8)
        xt_b = np.ascontiguousarray(x[b].T)                   # [512, 2048]
        xt_cc = np.stack([xt_b[:, cc * 512:(cc + 1) * 512] for cc in range(4)])
        x0 = xt_cc.astype(NP8)
        x1 = (xt_cc - x0.astype(np.float32)).astype(NP8)
        km = np.where(pm[b] != 0, 0.0, -1e5).astype(np.float32)
        in_maps.append({
            "xt0": np.ascontiguousarray(x0),
            "xt1": np.ascontiguousarray(x1),
            "wq0": np.ascontiguousarray(
                w0.reshape(4, 128, 384).transpose(1, 0, 2)),
            "wq1": np.ascontiguousarray(
                w1.reshape(4, 128, 384).transpose(1, 0, 2)),
            "wo": np.ascontiguousarray(
                Wo[g * 128:(g + 1) * 128, :]).astype(NPBF),
            "km": np.ascontiguousarray(km.reshape(NB, 128).T, dtype=np.float32),
            "tm": tm,
        })

    try:
        LAST_RESULTS = run_bass_kernel_spmd(nc, in_maps, core_ids=list(range(8)))
    except Exception:
        # transient device faults (e.g. NRT_EXEC_UNIT_UNRECOVERABLE) have been
        # observed to clear on the next attempt; retry once before giving up
        LAST_RESULTS = run_bass_kernel_spmd(nc, in_maps, core_ids=list(range(8)))
    res = LAST_RESULTS.results

    out = np.zeros((B, S, E), np.float32)
    for core in range(8):
        b = core // 4
        out[b] += np.asarray(res[core]["outt"]).astype(np.float32).T
    out *= (1.0 / 32.0)  # v (and hence vals) carry the 32x weight scale
    out += bo
    return out
